# revision 9
# baseline (speedup 1.0000x reference)
"""Trainium2 Bass kernel for the NoisyTopK MoE layer (B=2,T=2048,D=1024,H=4096,O=1024,E=8,K=2).

Strategy (expert-parallel, 8 cores = 8 experts):
  * Host: compute the full noisy-top2 routing (indices AND softmax gates,
    tiny numpy), gather each expert's tokens, pad to a common capacity C
    (= the max expert load, exactly).
  * Device (per core, SPMD — same program, per-expert data):
      expert FFN: out = (relu(x @ W1 + b1) @ W2 + b2) * gate
      fused MM1->MM2 per H-slice, f16 matmuls, W1 AND W2 resident in SBUF
      (16MB total; W1 streamed in during chunk 0 via the Scalar engine's
      HWDGE queue, W2 via Sync's), per-token gate scalar from host.
  * Host: scatter-add the per-expert outputs back to [B,T,O]
    (equivalent to the all-reduce of the gated combine).
"""

import os
import time

import numpy as np

P = 128
B, T, D, H, O, E = 2, 2048, 1024, 4096, 1024, 8
KD = D // P   # 8  k-tiles over D
KH = H // P   # 32 k-tiles over H (= number of m-slices of MM1)
NM = H // P   # 32 m-slices
OS = 2        # O-slices of 512
TB_PER_CHUNK = 3  # 384 tokens per chunk

_NC_CACHE = {}
LAST_RUN = {}


def _build_nc(C):
    import concourse.mybir as mybir
    import concourse.tile as tile
    from concourse import bacc

    f32 = mybir.dt.float32
    f16 = mybir.dt.float16
    AF = mybir.ActivationFunctionType

    NTB = (C + P - 1) // P  # token blocks; last may be partial
    blocks = [P] * (C // P) + ([C % P] if C % P else [])
    chunks = []  # (first block idx, global token offset, [block sizes])
    b0 = 0
    while b0 < NTB:
        n = min(TB_PER_CHUNK, NTB - b0)
        chunks.append((b0, sum(blocks[:b0]), blocks[b0 : b0 + n]))
        b0 += n

    # Bacc (not plain Bass): its compile() pass splits multi-wait matmuls
    # (HW allows a single sync-wait on the fused LDWEIGHTS+MATMULT).
    nc = bacc.Bacc()
    xh_d = nc.declare_dram_parameter("xh", [P, KD, C], f16, isOutput=False)
    w1_d = nc.declare_dram_parameter("w1s", [NM, P, KD, P], f16, isOutput=False)
    w2_d = nc.declare_dram_parameter("w2s", [P, KH, O], f16, isOutput=False)
    b1_d = nc.declare_dram_parameter("b1s", [P, NM], f32, isOutput=False)
    b2_d = nc.declare_dram_parameter("b2e", [1, O], f32, isOutput=False)
    g_d = nc.declare_dram_parameter("gates", [P, NTB], f32, isOutput=False)
    out_d = nc.declare_dram_parameter("out", [C, O], f32, isOutput=True)

    with tile.TileContext(nc) as tc:
        with (
            tc.tile_pool(name="singles", bufs=1) as singles,
            tc.tile_pool(name="xpool", bufs=2 * KD) as xpool,
            tc.tile_pool(name="hpool", bufs=8) as hpool,
            tc.tile_pool(name="spool", bufs=2) as spool,
            tc.tile_pool(name="psA", bufs=6, space="PSUM") as psA,
            tc.tile_pool(name="psB", bufs=2, space="PSUM") as psB,
        ):
            # ---- resident tensors ----
            # W1 (8MB) and W2 (8MB) both live in SBUF for the whole kernel;
            # their slices are loaded just-in-time inside chunk 0's m-loop
            # (W1 via the Scalar HWDGE queue, W2 via Sync's) so chunks 1+
            # run with zero weight DMA.
            w1_sb = singles.tile([P, NM, KD, P], f16)
            w2_sb = singles.tile([P, KH, O], f16)
            b1_sb = singles.tile([P, NM], f32)
            b2_sb = singles.tile([P, O], f32)
            g_sb = singles.tile([P, NTB], f32)

            def emit_w1_load(m):
                # 2-way partition split: two DMA queues move the 256KB in
                # parallel, halving the load latency
                for pr in (0, 64):
                    nc.scalar.dma_start(
                        w1_sb[pr : pr + 64, m], w1_d[m, pr : pr + 64]
                    )

            def emit_x_load(ci):
                # x for one chunk, one tile per ko: tile-granular deps let
                # MM1 ko=0 start as soon as its 96KB slice lands instead of
                # waiting for the whole chunk
                _, t0c, bsz = chunks[ci]
                nt = sum(bsz)
                xs = [
                    xpool.tile(
                        [P, TB_PER_CHUNK * P], f16, tag="xs", name=f"xs{ko}"
                    )
                    for ko in range(KD)
                ]
                for ko in range(KD):
                    nc.sync.dma_start(
                        xs[ko][:, :nt], xh_d[:, ko, t0c : t0c + nt]
                    )
                return xs

            # startup-critical emission order: each engine's DMA queue leads
            # with the loads the first matmul group needs
            emit_w1_load(0)      # Scalar queue: w1[0] halves
            xs_next = emit_x_load(0)  # Sync queue: xs ko 0..7
            emit_w1_load(1)
            emit_w1_load(2)
            nc.sync.dma_start(b1_sb[:], b1_d[:])

            def emit_setup_small():
                # evict-phase constants — deferred so they don't sit ahead
                # of the chunk-0 x/W1 loads in the DMA queues
                nc.sync.dma_start(g_sb[:], g_d[:])
                nc.sync.dma_start(b2_sb[:], b2_d[0].partition_broadcast(P))

            # MM2 trails MM1 by DELTA H-slices: the PE always has independent
            # MM1 work while MM2 waits on relu eviction / psum-slot release.
            DELTA = 6

            for ci, (b0c, t0c, bsz) in enumerate(chunks):
                nt = sum(bsz)
                ntb = len(bsz)
                bofs = [sum(bsz[:j]) for j in range(ntb)]
                xs = xs_next
                accs = [
                    [
                        psA.tile([P, 512], f32, tag="acc", name=f"acc_{j}_{osl}")
                        for osl in range(OS)
                    ]
                    for j in range(ntb)
                ]
                # a <128-wide final block would give MM2 a narrow stationary
                # (disables FWL, +50ns/MM measured); zero-pad hm so its MM2s
                # run as full 128-col stationary instead
                padw = (bofs[-1] + P) - nt if bsz[-1] < P else 0
                hms = {}
                for m in range(NM):
                    if ci == 0 and m == 4:
                        # deferred past the first MM1s so the DMA queues
                        # drain the critical-path loads first
                        emit_setup_small()
                    if m == 18 and ci + 1 < len(chunks):
                        # prefetch next chunk's x while this chunk's m-loop
                        # keeps the PE saturated
                        xs_next = emit_x_load(ci + 1)
                    if ci == 0:
                        if m + 3 < NM:
                            emit_w1_load(m + 3)
                        nc.sync.dma_start(w2_sb[:, m, :], w2_d[:, m, :])
                    hps = psB.tile([P, TB_PER_CHUNK * P], f32, tag="mm1ps")
                    hw = hps[:, :nt]
                    for ko in range(KD):
                        nc.tensor.matmul(
                            hw,
                            w1_sb[:, m, ko, :],
                            xs[ko][:, :nt],
                            start=(ko == 0),
                            stop=(ko == KD - 1),
                        )
                    hm = hpool.tile([P, TB_PER_CHUNK * P], f16, tag="hm")
                    nc.scalar.activation(
                        hm[:, :nt], hw, AF.Relu, bias=b1_sb[:, m : m + 1]
                    )
                    if padw:
                        nc.vector.memset(hm[:, nt : nt + padw], 0.0)
                    hms[m] = hm
                    if m >= DELTA:
                        mm = m - DELTA
                        hm2 = hms.pop(mm)
                        for j in range(ntb):
                            bs = P if j == ntb - 1 and padw else bsz[j]
                            for osl in range(OS):
                                nc.tensor.matmul(
                                    accs[j][osl][:bs],
                                    hm2[:, bofs[j] : bofs[j] + bs],
                                    w2_sb[:, mm, osl * 512 : (osl + 1) * 512],
                                    start=(mm == 0),
                                    stop=(mm == NM - 1),
                                )

                # ---- pipeline drain, block-major: finish block j's
                # accumulation, then evict it while block j+1 drains ----
                for j in range(ntb):
                    bs = bsz[j]
                    bsm = P if j == ntb - 1 and padw else bs
                    for mm in range(NM - DELTA, NM):
                        hm2 = hms[mm]
                        for osl in range(OS):
                            nc.tensor.matmul(
                                accs[j][osl][:bsm],
                                hm2[:, bofs[j] : bofs[j] + bsm],
                                w2_sb[:, mm, osl * 512 : (osl + 1) * 512],
                                start=(mm == 0),
                                stop=(mm == NM - 1),
                            )
                    # evict: (acc + b2) * gate -> DRAM
                    st = spool.tile([P, O], f32, tag="st")
                    for osl in range(OS):
                        sl = slice(osl * 512, (osl + 1) * 512)
                        nc.vector.tensor_add(
                            st[:bs, sl], accs[j][osl][:bs], b2_sb[:bs, sl]
                        )
                        nc.vector.tensor_scalar_mul(
                            st[:bs, sl],
                            st[:bs, sl],
                            g_sb[:bs, b0c + j : b0c + j + 1],
                        )
                    g0 = t0c + bofs[j]
                    nc.sync.dma_start(out_d[g0 : g0 + bs, :], st[:bs, :])
                hms.clear()

    nc.finalize()
    return nc


def _routing_host(xf, nf, Wg, bg, Wn, bn):
    """Top-2 expert mask AND the sparse softmax gates per token."""
    logits = xf @ Wg + bg
    nl = xf @ Wn + bn
    sp = np.logaddexp(0.0, nl)
    noisy = logits + nf * sp
    order = np.argpartition(-noisy, 2, axis=1)[:, :2]
    mask = np.zeros(noisy.shape, dtype=bool)
    mask[np.arange(noisy.shape[0])[:, None], order] = True
    # softmax over the two selected logits (matches reference: softmax of
    # the -inf-masked logits, then L1-normalize — a numeric no-op)
    neg = np.where(mask, noisy, -np.inf)
    mx = neg.max(axis=1, keepdims=True)
    ex = np.exp(neg - mx)
    gates = ex / ex.sum(axis=1, keepdims=True)
    gates[~mask] = 0.0
    return mask, gates.astype(np.float32)


def _prep_core(xf, gates, idx, C, W1e, b1e, W2e, b2e, e):
    n = len(idx)
    x_g = np.zeros((C, D), np.float32)
    x_g[:n] = xf[idx]
    NTB = (C + P - 1) // P
    g_g = np.zeros((NTB * P,), np.float32)
    g_g[:n] = gates[idx, e]
    xh = np.ascontiguousarray(
        x_g.reshape(C, KD, P).transpose(2, 1, 0)
    ).astype(np.float16)
    return {
        "xh": xh,
        "w1s": np.ascontiguousarray(
            W1e.reshape(KD, P, NM, P).transpose(2, 1, 0, 3)
        ).astype(np.float16),
        "w2s": np.ascontiguousarray(
            W2e.reshape(KH, P, O).transpose(1, 0, 2)
        ).astype(np.float16),
        "b1s": np.ascontiguousarray(b1e.reshape(NM, P).T),
        "b2e": b2e[None, :].astype(np.float32),
        "gates": np.ascontiguousarray(g_g.reshape(NTB, P).T),
    }


def kernel(x, noise, Wg, bg, Wn, bn, W1, b1, W2, b2):
    from concourse.bass_utils import run_bass_kernel_spmd

    x = np.asarray(x, np.float32)
    noise = np.asarray(noise, np.float32)
    Wg = np.asarray(Wg, np.float32)
    bg = np.asarray(bg, np.float32)
    Wn = np.asarray(Wn, np.float32)
    bn = np.asarray(bn, np.float32)
    W1 = np.asarray(W1, np.float32)
    b1 = np.asarray(b1, np.float32)
    W2 = np.asarray(W2, np.float32)
    b2 = np.asarray(b2, np.float32)

    Bx, Tx, _ = x.shape
    ntok = Bx * Tx
    xf = x.reshape(ntok, D)
    nf = noise.reshape(ntok, E)

    mask, gates = _routing_host(xf, nf, Wg, bg, Wn, bn)
    idx = [np.nonzero(mask[:, e])[0] for e in range(E)]
    C = max(2 * P, max(len(i) for i in idx))

    if C not in _NC_CACHE:
        _NC_CACHE[C] = _build_nc(C)
    nc = _NC_CACHE[C]

    in_maps = [
        _prep_core(xf, gates, idx[e], C, W1[e], b1[e], W2[e], b2[e], e)
        for e in range(E)
    ]

    trace = bool(os.environ.get("MOE_TRACE"))
    t0 = time.time()
    res = run_bass_kernel_spmd(
        nc, in_maps, list(range(E)), trace=trace
    )
    t1 = time.time()
    LAST_RUN.clear()
    LAST_RUN.update(
        wall_s=t1 - t0,
        exec_time_ns=res.exec_time_ns,
        trace=res.instructions_and_trace[1] if res.instructions_and_trace else None,
    )

    out = np.zeros((ntok, O), np.float32)
    for e in range(E):
        n = len(idx[e])
        y = res.results[e]["out"].reshape(C, O)
        out[idx[e]] += y[:n]
    return out.reshape(Bx, Tx, O)


# revision 11
# speedup vs baseline: 1.0651x; 1.0651x over previous
"""Trainium2 Bass kernel for the NoisyTopK MoE layer (B=2,T=2048,D=1024,H=4096,O=1024,E=8,K=2).

Strategy (expert-parallel, 8 cores = 8 experts):
  * Host: compute the full noisy-top2 routing (indices AND softmax gates,
    tiny numpy), gather each expert's tokens, pad to a common capacity C
    (= the max expert load, exactly).
  * Device (per core, SPMD — same program, per-expert data):
      expert FFN: out = (relu(x @ W1 + b1) @ W2 + b2) * gate
      fused MM1->MM2 per H-slice, f16 matmuls, W1 AND W2 resident in SBUF
      (16MB total; W1 streamed in during chunk 0 via the Scalar engine's
      HWDGE queue, W2 via Sync's), per-token gate scalar from host.
  * Host: scatter-add the per-expert outputs back to [B,T,O]
    (equivalent to the all-reduce of the gated combine).
"""

import os
import time

import numpy as np

P = 128
B, T, D, H, O, E = 2, 2048, 1024, 4096, 1024, 8
KD = D // P   # 8  k-tiles over D
KH = H // P   # 32 k-tiles over H (= number of m-slices of MM1)
NM = H // P   # 32 m-slices
OS = 2        # O-slices of 512
TB_PER_CHUNK = 3  # 384 tokens per chunk

_NC_CACHE = {}
LAST_RUN = {}


def _build_nc(C):
    import concourse.mybir as mybir
    import concourse.tile as tile
    from concourse import bacc

    f32 = mybir.dt.float32
    f16 = mybir.dt.float16
    AF = mybir.ActivationFunctionType

    NTB = (C + P - 1) // P  # token blocks; last may be partial
    blocks = [P] * (C // P) + ([C % P] if C % P else [])
    chunks = []  # (first block idx, global token offset, [block sizes])
    b0 = 0
    while b0 < NTB:
        n = min(TB_PER_CHUNK, NTB - b0)
        chunks.append((b0, sum(blocks[:b0]), blocks[b0 : b0 + n]))
        b0 += n

    # Bacc (not plain Bass): its compile() pass splits multi-wait matmuls
    # (HW allows a single sync-wait on the fused LDWEIGHTS+MATMULT).
    nc = bacc.Bacc()
    xh_d = nc.declare_dram_parameter("xh", [P, KD, C], f16, isOutput=False)
    w1_d = nc.declare_dram_parameter("w1s", [NM, P, KD, P], f16, isOutput=False)
    w2_d = nc.declare_dram_parameter("w2s", [P, KH, O], f16, isOutput=False)
    b1_d = nc.declare_dram_parameter("b1s", [P, NM], f32, isOutput=False)
    b2_d = nc.declare_dram_parameter("b2e", [1, O], f32, isOutput=False)
    g_d = nc.declare_dram_parameter("gates", [P, NTB], f32, isOutput=False)
    out_d = nc.declare_dram_parameter("out", [C, O], f32, isOutput=True)

    with tile.TileContext(nc) as tc:
        with (
            tc.tile_pool(name="singles", bufs=1) as singles,
            tc.tile_pool(name="xpool", bufs=2 * KD) as xpool,
            tc.tile_pool(name="hpool", bufs=8) as hpool,
            tc.tile_pool(name="spool", bufs=2) as spool,
            tc.tile_pool(name="psA", bufs=6, space="PSUM") as psA,
            tc.tile_pool(name="psB", bufs=2, space="PSUM") as psB,
        ):
            # ---- resident tensors ----
            # W1 (8MB) and W2 (8MB) both live in SBUF for the whole kernel;
            # their slices are loaded just-in-time inside chunk 0's m-loop
            # (W1 via the Scalar HWDGE queue, W2 via Sync's) so chunks 1+
            # run with zero weight DMA.
            w1_sb = singles.tile([P, NM, KD, P], f16)
            w2_sb = singles.tile([P, KH, O], f16)
            b1_sb = singles.tile([P, NM], f32)
            b2_sb = singles.tile([P, O], f32)
            g_sb = singles.tile([P, NTB], f32)

            def emit_w1_load(m, split=False):
                # w1 and w2 share the Sync issue queue, interleaved per m:
                # issue order is the pacing that keeps the 16 physical DMA
                # engines fair between the two streams (a separate Scalar
                # ring let w2 hog the engines and starved MM1)
                if split:
                    # 2-way partition split halves latency when queues are
                    # empty (startup)
                    for pr in (0, 64):
                        nc.sync.dma_start(
                            w1_sb[pr : pr + 64, m], w1_d[m, pr : pr + 64]
                        )
                else:
                    nc.sync.dma_start(w1_sb[:, m], w1_d[m])

            def emit_x_load(ci):
                # x for one chunk, one tile per ko: tile-granular deps let
                # MM1 ko=0 start as soon as its 96KB slice lands instead of
                # waiting for the whole chunk
                _, t0c, bsz = chunks[ci]
                nt = sum(bsz)
                xs = [
                    xpool.tile(
                        [P, TB_PER_CHUNK * P], f16, tag="xs", name=f"xs{ko}"
                    )
                    for ko in range(KD)
                ]
                for ko in range(KD):
                    nc.sync.dma_start(
                        xs[ko][:, :nt], xh_d[:, ko, t0c : t0c + nt]
                    )
                return xs

            # startup-critical emission order: the queue leads with the
            # loads the first matmul group needs
            emit_w1_load(0, split=True)
            xs_next = emit_x_load(0)
            emit_w1_load(1, split=True)
            emit_w1_load(2)
            nc.sync.dma_start(b1_sb[:], b1_d[:])

            def emit_setup_small():
                # evict-phase constants — deferred so they don't sit ahead
                # of the chunk-0 x/W1 loads in the DMA queues
                nc.sync.dma_start(g_sb[:], g_d[:])
                nc.sync.dma_start(b2_sb[:], b2_d[0].partition_broadcast(P))

            # MM2 trails MM1 by DELTA H-slices: the PE always has independent
            # MM1 work while MM2 waits on relu eviction / psum-slot release.
            DELTA = 6

            for ci, (b0c, t0c, bsz) in enumerate(chunks):
                nt = sum(bsz)
                ntb = len(bsz)
                bofs = [sum(bsz[:j]) for j in range(ntb)]
                xs = xs_next
                accs = [
                    [
                        psA.tile([P, 512], f32, tag="acc", name=f"acc_{j}_{osl}")
                        for osl in range(OS)
                    ]
                    for j in range(ntb)
                ]
                # a <128-wide final block would give MM2 a narrow stationary
                # (disables FWL, +50ns/MM measured); zero-pad hm so its MM2s
                # run as full 128-col stationary instead
                padw = (bofs[-1] + P) - nt if bsz[-1] < P else 0
                hms = {}
                for m in range(NM):
                    if ci == 0 and m == 4:
                        # deferred past the first MM1s so the DMA queues
                        # drain the critical-path loads first
                        emit_setup_small()
                    if m == 18 and ci + 1 < len(chunks):
                        # prefetch next chunk's x while this chunk's m-loop
                        # keeps the PE saturated
                        xs_next = emit_x_load(ci + 1)
                    if ci == 0:
                        if m + 3 < NM:
                            emit_w1_load(m + 3)
                        nc.sync.dma_start(w2_sb[:, m, :], w2_d[:, m, :])
                    hps = psB.tile([P, TB_PER_CHUNK * P], f32, tag="mm1ps")
                    hw = hps[:, :nt]
                    for ko in range(KD):
                        nc.tensor.matmul(
                            hw,
                            w1_sb[:, m, ko, :],
                            xs[ko][:, :nt],
                            start=(ko == 0),
                            stop=(ko == KD - 1),
                        )
                    hm = hpool.tile([P, TB_PER_CHUNK * P], f16, tag="hm")
                    nc.scalar.activation(
                        hm[:, :nt], hw, AF.Relu, bias=b1_sb[:, m : m + 1]
                    )
                    if padw:
                        nc.vector.memset(hm[:, nt : nt + padw], 0.0)
                    hms[m] = hm
                    if m >= DELTA:
                        mm = m - DELTA
                        hm2 = hms.pop(mm)
                        for j in range(ntb):
                            bs = P if j == ntb - 1 and padw else bsz[j]
                            for osl in range(OS):
                                nc.tensor.matmul(
                                    accs[j][osl][:bs],
                                    hm2[:, bofs[j] : bofs[j] + bs],
                                    w2_sb[:, mm, osl * 512 : (osl + 1) * 512],
                                    start=(mm == 0),
                                    stop=(mm == NM - 1),
                                )

                # ---- pipeline drain, block-major: finish block j's
                # accumulation, then evict it while block j+1 drains ----
                for j in range(ntb):
                    bs = bsz[j]
                    bsm = P if j == ntb - 1 and padw else bs
                    for mm in range(NM - DELTA, NM):
                        hm2 = hms[mm]
                        for osl in range(OS):
                            nc.tensor.matmul(
                                accs[j][osl][:bsm],
                                hm2[:, bofs[j] : bofs[j] + bsm],
                                w2_sb[:, mm, osl * 512 : (osl + 1) * 512],
                                start=(mm == 0),
                                stop=(mm == NM - 1),
                            )
                    # evict: (acc + b2) * gate -> DRAM
                    st = spool.tile([P, O], f32, tag="st")
                    for osl in range(OS):
                        sl = slice(osl * 512, (osl + 1) * 512)
                        nc.vector.tensor_add(
                            st[:bs, sl], accs[j][osl][:bs], b2_sb[:bs, sl]
                        )
                        nc.vector.tensor_scalar_mul(
                            st[:bs, sl],
                            st[:bs, sl],
                            g_sb[:bs, b0c + j : b0c + j + 1],
                        )
                    g0 = t0c + bofs[j]
                    nc.sync.dma_start(out_d[g0 : g0 + bs, :], st[:bs, :])
                hms.clear()

    nc.finalize()
    return nc


def _routing_host(xf, nf, Wg, bg, Wn, bn):
    """Top-2 expert mask AND the sparse softmax gates per token."""
    logits = xf @ Wg + bg
    nl = xf @ Wn + bn
    sp = np.logaddexp(0.0, nl)
    noisy = logits + nf * sp
    order = np.argpartition(-noisy, 2, axis=1)[:, :2]
    mask = np.zeros(noisy.shape, dtype=bool)
    mask[np.arange(noisy.shape[0])[:, None], order] = True
    # softmax over the two selected logits (matches reference: softmax of
    # the -inf-masked logits, then L1-normalize — a numeric no-op)
    neg = np.where(mask, noisy, -np.inf)
    mx = neg.max(axis=1, keepdims=True)
    ex = np.exp(neg - mx)
    gates = ex / ex.sum(axis=1, keepdims=True)
    gates[~mask] = 0.0
    return mask, gates.astype(np.float32)


def _prep_core(xf, gates, idx, C, W1e, b1e, W2e, b2e, e):
    n = len(idx)
    x_g = np.zeros((C, D), np.float32)
    x_g[:n] = xf[idx]
    NTB = (C + P - 1) // P
    g_g = np.zeros((NTB * P,), np.float32)
    g_g[:n] = gates[idx, e]
    xh = np.ascontiguousarray(
        x_g.reshape(C, KD, P).transpose(2, 1, 0)
    ).astype(np.float16)
    return {
        "xh": xh,
        "w1s": np.ascontiguousarray(
            W1e.reshape(KD, P, NM, P).transpose(2, 1, 0, 3)
        ).astype(np.float16),
        "w2s": np.ascontiguousarray(
            W2e.reshape(KH, P, O).transpose(1, 0, 2)
        ).astype(np.float16),
        "b1s": np.ascontiguousarray(b1e.reshape(NM, P).T),
        "b2e": b2e[None, :].astype(np.float32),
        "gates": np.ascontiguousarray(g_g.reshape(NTB, P).T),
    }


def kernel(x, noise, Wg, bg, Wn, bn, W1, b1, W2, b2):
    from concourse.bass_utils import run_bass_kernel_spmd

    x = np.asarray(x, np.float32)
    noise = np.asarray(noise, np.float32)
    Wg = np.asarray(Wg, np.float32)
    bg = np.asarray(bg, np.float32)
    Wn = np.asarray(Wn, np.float32)
    bn = np.asarray(bn, np.float32)
    W1 = np.asarray(W1, np.float32)
    b1 = np.asarray(b1, np.float32)
    W2 = np.asarray(W2, np.float32)
    b2 = np.asarray(b2, np.float32)

    Bx, Tx, _ = x.shape
    ntok = Bx * Tx
    xf = x.reshape(ntok, D)
    nf = noise.reshape(ntok, E)

    mask, gates = _routing_host(xf, nf, Wg, bg, Wn, bn)
    idx = [np.nonzero(mask[:, e])[0] for e in range(E)]
    C = max(2 * P, max(len(i) for i in idx))

    if C not in _NC_CACHE:
        _NC_CACHE[C] = _build_nc(C)
    nc = _NC_CACHE[C]

    in_maps = [
        _prep_core(xf, gates, idx[e], C, W1[e], b1[e], W2[e], b2[e], e)
        for e in range(E)
    ]

    trace = bool(os.environ.get("MOE_TRACE"))
    t0 = time.time()
    res = run_bass_kernel_spmd(
        nc, in_maps, list(range(E)), trace=trace
    )
    t1 = time.time()
    LAST_RUN.clear()
    LAST_RUN.update(
        wall_s=t1 - t0,
        exec_time_ns=res.exec_time_ns,
        trace=res.instructions_and_trace[1] if res.instructions_and_trace else None,
    )

    out = np.zeros((ntok, O), np.float32)
    for e in range(E):
        n = len(idx[e])
        y = res.results[e]["out"].reshape(C, O)
        out[idx[e]] += y[:n]
    return out.reshape(Bx, Tx, O)


# revision 16
# speedup vs baseline: 1.0828x; 1.0167x over previous
"""Trainium2 Bass kernel for the NoisyTopK MoE layer (B=2,T=2048,D=1024,H=4096,O=1024,E=8,K=2).

Strategy (expert-parallel, 8 cores = 8 experts):
  * Host: compute the full noisy-top2 routing (indices AND softmax gates,
    tiny numpy), gather each expert's tokens, pad to a common capacity C
    (= the max expert load, exactly).
  * Device (per core, SPMD — same program, per-expert data):
      expert FFN: out = (relu(x @ W1 + b1) @ W2 + b2) * gate
      fused MM1->MM2 per H-slice, f16 matmuls, W1 AND W2 resident in SBUF
      (16MB total; W1 streamed in during chunk 0 via the Scalar engine's
      HWDGE queue, W2 via Sync's), per-token gate scalar from host.
  * Host: scatter-add the per-expert outputs back to [B,T,O]
    (equivalent to the all-reduce of the gated combine).
"""

import os
import time

import numpy as np

P = 128
B, T, D, H, O, E = 2, 2048, 1024, 4096, 1024, 8
KD = D // P   # 8  k-tiles over D
KH = H // P   # 32 k-tiles over H (= number of m-slices of MM1)
NM = H // P   # 32 m-slices
OS = 2        # O-slices of 512
TB_PER_CHUNK = 3  # 384 tokens per chunk

_NC_CACHE = {}
LAST_RUN = {}


def _build_nc(C):
    import concourse.mybir as mybir
    import concourse.tile as tile
    from concourse import bacc

    f32 = mybir.dt.float32
    f16 = mybir.dt.float16
    AF = mybir.ActivationFunctionType

    NTB = (C + P - 1) // P  # token blocks; last may be partial
    blocks = [P] * (C // P) + ([C % P] if C % P else [])
    chunks = []  # (first block idx, global token offset, [block sizes])
    b0 = 0
    while b0 < NTB:
        n = min(TB_PER_CHUNK, NTB - b0)
        chunks.append((b0, sum(blocks[:b0]), blocks[b0 : b0 + n]))
        b0 += n

    # Bacc (not plain Bass): its compile() pass splits multi-wait matmuls
    # (HW allows a single sync-wait on the fused LDWEIGHTS+MATMULT).
    nc = bacc.Bacc()
    xh_d = nc.declare_dram_parameter("xh", [P, KD, C], f16, isOutput=False)
    w1_d = nc.declare_dram_parameter("w1s", [NM, P, KD, P], f16, isOutput=False)
    w2_d = nc.declare_dram_parameter("w2s", [P, KH, O], f16, isOutput=False)
    b1_d = nc.declare_dram_parameter("b1s", [P, NM], f32, isOutput=False)
    b2_d = nc.declare_dram_parameter("b2e", [1, O], f32, isOutput=False)
    g_d = nc.declare_dram_parameter("gates", [P, NTB], f32, isOutput=False)
    # f16 output: halves store bytes so the final block's store doesn't
    # trail the last matmul by ~9us; quantization (~6e-4 of out scale) is
    # far under the accuracy budget
    out_d = nc.declare_dram_parameter("out", [C, O], f16, isOutput=True)

    with tile.TileContext(nc) as tc:
        with (
            tc.tile_pool(name="singles", bufs=1) as singles,
            tc.tile_pool(name="xpool", bufs=2 * KD) as xpool,
            tc.tile_pool(name="hpool", bufs=8) as hpool,
            tc.tile_pool(name="spool", bufs=2) as spool,
            tc.tile_pool(name="psA", bufs=6, space="PSUM") as psA,
            tc.tile_pool(name="psB", bufs=2, space="PSUM") as psB,
        ):
            # ---- resident tensors ----
            # W1 (8MB) and W2 (8MB) both live in SBUF for the whole kernel;
            # their slices are loaded just-in-time inside chunk 0's m-loop
            # (W1 via the Scalar HWDGE queue, W2 via Sync's) so chunks 1+
            # run with zero weight DMA.
            w1_sb = singles.tile([P, NM, KD, P], f16)
            w2_sb = singles.tile([P, KH, O], f16)
            b1_sb = singles.tile([P, NM], f32)
            b2_sb = singles.tile([P, O], f32)
            g_sb = singles.tile([P, NTB], f32)

            def emit_w1_load(m, split=False):
                # w1 and w2 share the Sync issue queue, interleaved per m:
                # issue order is the pacing that keeps the 16 physical DMA
                # engines fair between the two streams (a separate Scalar
                # ring let w2 hog the engines and starved MM1)
                if split:
                    # 2-way partition split halves latency when queues are
                    # empty (startup)
                    for pr in (0, 64):
                        nc.sync.dma_start(
                            w1_sb[pr : pr + 64, m], w1_d[m, pr : pr + 64]
                        )
                else:
                    nc.sync.dma_start(w1_sb[:, m], w1_d[m])

            def emit_x_load(ci):
                # x for one chunk, one tile per ko: tile-granular deps let
                # MM1 ko=0 start as soon as its 96KB slice lands instead of
                # waiting for the whole chunk
                _, t0c, bsz = chunks[ci]
                nt = sum(bsz)
                xs = [
                    xpool.tile(
                        [P, TB_PER_CHUNK * P], f16, tag="xs", name=f"xs{ko}"
                    )
                    for ko in range(KD)
                ]
                for ko in range(KD):
                    nc.sync.dma_start(
                        xs[ko][:, :nt], xh_d[:, ko, t0c : t0c + nt]
                    )
                return xs

            # startup-critical emission order: the queue leads with the
            # loads the first matmul group needs
            emit_w1_load(0, split=True)
            xs_next = emit_x_load(0)
            emit_w1_load(1, split=True)
            emit_w1_load(2, split=True)
            nc.sync.dma_start(b1_sb[:], b1_d[:])

            def emit_setup_small():
                # evict-phase constants — deferred so they don't sit ahead
                # of the chunk-0 x/W1 loads in the DMA queues
                nc.sync.dma_start(g_sb[:], g_d[:])
                nc.sync.dma_start(b2_sb[:], b2_d[0].partition_broadcast(P))

            # MM2 trails MM1 by DELTA H-slices: the PE always has independent
            # MM1 work while MM2 waits on relu eviction / psum-slot release.
            DELTA = 6

            for ci, (b0c, t0c, bsz) in enumerate(chunks):
                nt = sum(bsz)
                ntb = len(bsz)
                bofs = [sum(bsz[:j]) for j in range(ntb)]
                xs = xs_next
                accs = [
                    [
                        psA.tile([P, 512], f32, tag="acc", name=f"acc_{j}_{osl}")
                        for osl in range(OS)
                    ]
                    for j in range(ntb)
                ]
                # a <128-wide final block would give MM2 a narrow stationary
                # (disables FWL, +50ns/MM measured); zero-pad hm so its MM2s
                # run as full 128-col stationary instead
                padw = (bofs[-1] + P) - nt if bsz[-1] < P else 0
                hms = {}
                for m in range(NM):
                    if ci == 0 and m == 4:
                        # deferred past the first MM1s so the DMA queues
                        # drain the critical-path loads first
                        emit_setup_small()
                    if m == 18 and ci + 1 < len(chunks):
                        # prefetch next chunk's x while this chunk's m-loop
                        # keeps the PE saturated
                        xs_next = emit_x_load(ci + 1)
                    if ci == 0:
                        if m + 3 < NM:
                            emit_w1_load(m + 3, split=(m + 3 <= 6))
                        nc.sync.dma_start(w2_sb[:, m, :], w2_d[:, m, :])
                    hps = psB.tile([P, TB_PER_CHUNK * P], f32, tag="mm1ps")
                    hw = hps[:, :nt]
                    for ko in range(KD):
                        nc.tensor.matmul(
                            hw,
                            w1_sb[:, m, ko, :],
                            xs[ko][:, :nt],
                            start=(ko == 0),
                            stop=(ko == KD - 1),
                        )
                    hm = hpool.tile([P, TB_PER_CHUNK * P], f16, tag="hm")
                    nc.scalar.activation(
                        hm[:, :nt], hw, AF.Relu, bias=b1_sb[:, m : m + 1]
                    )
                    if padw:
                        nc.vector.memset(hm[:, nt : nt + padw], 0.0)
                    hms[m] = hm
                    if m >= DELTA:
                        mm = m - DELTA
                        hm2 = hms.pop(mm)
                        for j in range(ntb):
                            bs = P if j == ntb - 1 and padw else bsz[j]
                            for osl in range(OS):
                                nc.tensor.matmul(
                                    accs[j][osl][:bs],
                                    hm2[:, bofs[j] : bofs[j] + bs],
                                    w2_sb[:, mm, osl * 512 : (osl + 1) * 512],
                                    start=(mm == 0),
                                    stop=(mm == NM - 1),
                                )

                # ---- pipeline drain, block-major: finish block j's
                # accumulation, then evict it while block j+1 drains ----
                for j in range(ntb):
                    bs = bsz[j]
                    bsm = P if j == ntb - 1 and padw else bs
                    for mm in range(NM - DELTA, NM):
                        hm2 = hms[mm]
                        for osl in range(OS):
                            nc.tensor.matmul(
                                accs[j][osl][:bsm],
                                hm2[:, bofs[j] : bofs[j] + bsm],
                                w2_sb[:, mm, osl * 512 : (osl + 1) * 512],
                                start=(mm == 0),
                                stop=(mm == NM - 1),
                            )
                    # evict: (acc + b2) * gate -> DRAM (f16, 2-queue split)
                    st = spool.tile([P, O], f16, tag="st")
                    for osl in range(OS):
                        sl = slice(osl * 512, (osl + 1) * 512)
                        nc.vector.tensor_add(
                            st[:bs, sl], accs[j][osl][:bs], b2_sb[:bs, sl]
                        )
                        nc.vector.tensor_scalar_mul(
                            st[:bs, sl],
                            st[:bs, sl],
                            g_sb[:bs, b0c + j : b0c + j + 1],
                        )
                    g0 = t0c + bofs[j]
                    h1 = bs // 2
                    nc.sync.dma_start(out_d[g0 : g0 + h1, :], st[:h1, :])
                    nc.sync.dma_start(
                        out_d[g0 + h1 : g0 + bs, :], st[h1:bs, :]
                    )
                hms.clear()

    nc.finalize()
    return nc


def _routing_host(xf, nf, Wg, bg, Wn, bn):
    """Top-2 expert mask AND the sparse softmax gates per token."""
    logits = xf @ Wg + bg
    nl = xf @ Wn + bn
    sp = np.logaddexp(0.0, nl)
    noisy = logits + nf * sp
    order = np.argpartition(-noisy, 2, axis=1)[:, :2]
    mask = np.zeros(noisy.shape, dtype=bool)
    mask[np.arange(noisy.shape[0])[:, None], order] = True
    # softmax over the two selected logits (matches reference: softmax of
    # the -inf-masked logits, then L1-normalize — a numeric no-op)
    neg = np.where(mask, noisy, -np.inf)
    mx = neg.max(axis=1, keepdims=True)
    ex = np.exp(neg - mx)
    gates = ex / ex.sum(axis=1, keepdims=True)
    gates[~mask] = 0.0
    return mask, gates.astype(np.float32)


def _prep_core(xf, gates, idx, C, W1e, b1e, W2e, b2e, e):
    n = len(idx)
    x_g = np.zeros((C, D), np.float32)
    x_g[:n] = xf[idx]
    NTB = (C + P - 1) // P
    g_g = np.zeros((NTB * P,), np.float32)
    g_g[:n] = gates[idx, e]
    xh = np.ascontiguousarray(
        x_g.reshape(C, KD, P).transpose(2, 1, 0)
    ).astype(np.float16)
    return {
        "xh": xh,
        "w1s": np.ascontiguousarray(
            W1e.reshape(KD, P, NM, P).transpose(2, 1, 0, 3)
        ).astype(np.float16),
        "w2s": np.ascontiguousarray(
            W2e.reshape(KH, P, O).transpose(1, 0, 2)
        ).astype(np.float16),
        "b1s": np.ascontiguousarray(b1e.reshape(NM, P).T),
        "b2e": b2e[None, :].astype(np.float32),
        "gates": np.ascontiguousarray(g_g.reshape(NTB, P).T),
    }


def kernel(x, noise, Wg, bg, Wn, bn, W1, b1, W2, b2):
    from concourse.bass_utils import run_bass_kernel_spmd

    x = np.asarray(x, np.float32)
    noise = np.asarray(noise, np.float32)
    Wg = np.asarray(Wg, np.float32)
    bg = np.asarray(bg, np.float32)
    Wn = np.asarray(Wn, np.float32)
    bn = np.asarray(bn, np.float32)
    W1 = np.asarray(W1, np.float32)
    b1 = np.asarray(b1, np.float32)
    W2 = np.asarray(W2, np.float32)
    b2 = np.asarray(b2, np.float32)

    Bx, Tx, _ = x.shape
    ntok = Bx * Tx
    xf = x.reshape(ntok, D)
    nf = noise.reshape(ntok, E)

    mask, gates = _routing_host(xf, nf, Wg, bg, Wn, bn)
    idx = [np.nonzero(mask[:, e])[0] for e in range(E)]
    C = max(2 * P, max(len(i) for i in idx))

    if C not in _NC_CACHE:
        _NC_CACHE[C] = _build_nc(C)
    nc = _NC_CACHE[C]

    in_maps = [
        _prep_core(xf, gates, idx[e], C, W1[e], b1[e], W2[e], b2[e], e)
        for e in range(E)
    ]

    trace = bool(os.environ.get("MOE_TRACE"))
    t0 = time.time()
    res = run_bass_kernel_spmd(
        nc, in_maps, list(range(E)), trace=trace
    )
    t1 = time.time()
    LAST_RUN.clear()
    LAST_RUN.update(
        wall_s=t1 - t0,
        exec_time_ns=res.exec_time_ns,
        trace=res.instructions_and_trace[1] if res.instructions_and_trace else None,
    )

    out = np.zeros((ntok, O), np.float32)
    for e in range(E):
        n = len(idx[e])
        y = res.results[e]["out"].reshape(C, O)
        out[idx[e]] += y[:n].astype(np.float32)
    return out.reshape(Bx, Tx, O)
